# revision 10
# baseline (speedup 1.0000x reference)
"""DigitCaps (dead-code-routing collapsed) Trainium2 Bass kernel.

Math (faithful to the reference):
    s[j,d]  = (1/512) * sum_{i,k} W[0,i,j,d,k] * x[i,k]      (10,16)
    sq      = s^2                                             (elementwise; last axis is size 1)
    out     = (sq/(1+sq)) * s/(sqrt(sq+EPS)+EPS)              (1,1,10,16,1)

Sharding: the 16-wide output dim `d` is split across 8 cores (2 each). Each
core reads its own 1/8 slice of W (320 KB) and computes its 20 outputs fully;
no cross-core reduction is needed. Host-side work is only slicing/packing of
inputs and concatenation of the 8 disjoint output slices.

Per-core device program:
    DMA in a packed [128, 673] f32 buffer:
        cols 0:640   W slice, laid out [p, (t, n, k)] where the contraction
                     index q = (i,k) is split as i = t*128 + p, n = j*2+dd
        cols 640:672 x laid out [p, (t, k)]
        col  672     the constant 1/512 (matmul lhsT column; folds the mean)
    DVE: T[p,t,n,k] = W[p,t,n,k] * x[p,t,k]   (stride-0 broadcast over n)
    PE:  4 accumulating matmuls (ones/512 column as lhsT) -> psum[1, (n,k)]
    DVE: reduce over k -> s[1, 20]; then the squash chain; DMA out 20 floats.
"""

import os
import sys
from contextlib import ExitStack

import numpy as np

for _p in ("/opt/trn_rl_repo", "/root/.axon_site/_ro/trn_rl_repo"):
    if os.path.isdir(_p) and _p not in sys.path:
        sys.path.append(_p)

N_IN, N_OUT, D_IN, D_OUT = 512, 10, 8, 16
EPS = 1e-7
N_CORES = 8
D_PER = D_OUT // N_CORES      # 2 output dims per core
N_PER = N_OUT * D_PER         # 20 outputs per core
P = 128                       # partitions
T = N_IN // P                 # 4 i-chunks
K = D_IN                      # 8
W_COLS = T * N_PER * K        # 640
X_OFF = W_COLS                # 640
X_COLS = T * K                # 32
TOT = X_OFF + X_COLS          # 672

_built = None
last_results = None           # BassKernelResults of the most recent run


def _build_nc():
    import concourse.bass as bass
    import concourse.tile as tile
    from concourse import bacc, mybir

    nc = bacc.Bacc("TRN2", num_devices=N_CORES)
    inp = nc.dram_tensor("inp", (P, TOT), mybir.dt.float32, kind="ExternalInput")
    out = nc.dram_tensor("out", (1, N_PER), mybir.dt.float32, kind="ExternalOutput")

    f32 = mybir.dt.float32
    with tile.TileContext(nc) as tc, ExitStack() as ctx:
        pool = ctx.enter_context(tc.tile_pool(name="p", bufs=1))
        pspool = ctx.enter_context(tc.tile_pool(name="ps", bufs=1, space="PSUM"))

        buf = pool.tile([P, TOT], f32)
        nc.sync.dma_start(out=buf, in_=inp[:, :])

        # stationary 1/512 column (memset on DVE so the matmul's lhsT and rhs
        # deps ride the same semaphore — walrus limits waits per Matmult)
        ones = pool.tile([P, 1], f32)
        nc.vector.memset(ones, 1.0 / N_IN)

        # T[p, t, n, k] = W[p, t, n, k] * x[p, t, k]
        tmul = pool.tile([P, W_COLS], f32)
        x_sl = buf[:, X_OFF:TOT]
        x_b = bass.AP(
            tensor=x_sl.tensor,
            offset=x_sl.offset,
            ap=[x_sl.ap[0], [K, T], [0, N_PER], [1, K]],
        )
        w_4d = buf[:, 0:W_COLS].rearrange("p (t n k) -> p t n k", t=T, n=N_PER)
        t_4d = tmul.rearrange("p (t n k) -> p t n k", t=T, n=N_PER)
        nc.vector.tensor_tensor(t_4d, w_4d, x_b, op=mybir.AluOpType.mult)

        # psum[0, n*K+k] = (1/512) * sum_{p,t} T[p, t, n, k]
        ps = pspool.tile([1, N_PER * K], f32)
        for t in range(T):
            nc.tensor.matmul(
                ps[0:1, :],
                lhsT=ones[:, 0:1],
                rhs=tmul[:, t * N_PER * K : (t + 1) * N_PER * K],
                start=(t == 0),
                stop=(t == T - 1),
            )

        # s[1, n] = sum_k psum[1, n, k]
        s = pool.tile([1, N_PER], f32)
        nc.vector.tensor_reduce(
            s,
            ps[0:1, :].rearrange("p (n k) -> p n k", n=N_PER),
            axis=mybir.AxisListType.X,
            op=mybir.AluOpType.add,
        )

        # squash: out = (s*sq) / ((1+sq) * (sqrt(sq+EPS)+EPS))
        # sq on DVE (not ACT) so no instruction needs waits on two different
        # semaphores (walrus compute-instruction structs fit a single wait).
        eps_t = pool.tile([1, 1], f32)
        nc.vector.memset(eps_t, EPS)
        sq = pool.tile([1, N_PER], f32)
        nc.vector.tensor_mul(sq, s, s)
        r = pool.tile([1, N_PER], f32)
        nc.scalar.activation(
            r, sq, mybir.ActivationFunctionType.Sqrt, bias=eps_t[0:1, 0:1]
        )
        num = pool.tile([1, N_PER], f32)
        nc.vector.tensor_mul(num, s, sq)
        d1 = pool.tile([1, N_PER], f32)
        nc.vector.tensor_scalar_add(d1, sq, 1.0)
        d2 = pool.tile([1, N_PER], f32)
        nc.vector.tensor_scalar_add(d2, r, EPS)
        den = pool.tile([1, N_PER], f32)
        nc.vector.tensor_mul(den, d1, d2)
        rec = pool.tile([1, N_PER], f32)
        nc.vector.reciprocal(rec, den)
        q = pool.tile([1, N_PER], f32)
        nc.vector.tensor_mul(q, num, rec)

        nc.sync.dma_start(out=out[:, :], in_=q)
    nc.finalize()
    return nc


def kernel(x, W):
    global _built, last_results
    from concourse.bass_utils import run_bass_kernel_spmd

    if _built is None:
        _built = _build_nc()
    nc = _built

    x = np.ascontiguousarray(np.asarray(x, dtype=np.float32))
    W = np.ascontiguousarray(np.asarray(W, dtype=np.float32))

    # x[p, t*K + k] = x[t*128 + p, k]
    xr = x.reshape(T, P, K).transpose(1, 0, 2).reshape(P, X_COLS)
    base = np.empty((P, TOT), dtype=np.float32)
    base[:, X_OFF:TOT] = xr

    in_maps = []
    for c in range(N_CORES):
        Wc = W[0][:, :, D_PER * c : D_PER * (c + 1), :]     # (512, 10, 2, 8)
        Wr = (
            Wc.reshape(T, P, N_OUT, D_PER, K)
            .transpose(1, 0, 2, 3, 4)
            .reshape(P, W_COLS)
        )
        buf = base.copy()
        buf[:, :W_COLS] = Wr
        in_maps.append({"inp": buf})

    res = run_bass_kernel_spmd(nc, in_maps, core_ids=list(range(N_CORES)))
    last_results = res

    v = np.zeros((N_OUT, D_OUT), dtype=np.float32)
    for c in range(N_CORES):
        v[:, D_PER * c : D_PER * (c + 1)] = res.results[c]["out"].reshape(
            N_OUT, D_PER
        )
    return v.reshape(1, 1, N_OUT, D_OUT, 1)


# revision 15
# speedup vs baseline: 1.1199x; 1.1199x over previous
"""DigitCaps (dead-code-routing collapsed) Trainium2 Bass kernel.

Math (faithful to the reference):
    s[j,d]  = (1/512) * sum_{i,k} W[0,i,j,d,k] * x[i,k]      (10,16)
    sq      = s^2                                             (elementwise; last axis is size 1)
    out     = (sq/(1+sq)) * s/(sqrt(sq+EPS)+EPS)              (1,1,10,16,1)

Sharding: the 16-wide output dim `d` is split across 8 cores (2 each). Each
core reads its own 1/8 slice of W (320 KB) and computes its 20 outputs fully;
no cross-core reduction is needed. Host-side work is only slicing/packing of
inputs and concatenation of the 8 disjoint output slices.

Per-core device program:
    two DMAs (SP + ACT HWDGE rings), each bringing [x | W-half] so the first
    half's compute overlaps the second half's transfer:
        half h: x at [h*352, h*352+32), W chunks 2h..2h+1 at [h*352+32, ...)
        W laid out [p, (t, n, k)]: contraction q=(i,k), i = t*128 + p,
        n = j*2+dd
    DVE: T[p,t,n,k] = W[p,t,n,k] * x[p,t,k]   (stride-0 broadcast over n)
    PE:  4 accumulating matmuls (1/512 column as lhsT) -> psum[1, (n,k)]
    DVE: reduce over k -> s[1, 20]; squash chain; DMA out 20 floats.
"""

import os
import sys
from contextlib import ExitStack

import numpy as np

for _p in ("/opt/trn_rl_repo", "/root/.axon_site/_ro/trn_rl_repo"):
    if os.path.isdir(_p) and _p not in sys.path:
        sys.path.append(_p)

N_IN, N_OUT, D_IN, D_OUT = 512, 10, 8, 16
EPS = 1e-7
N_CORES = 8
D_PER = D_OUT // N_CORES      # 2 output dims per core
N_PER = N_OUT * D_PER         # 20 outputs per core
P = 128                       # partitions
T = N_IN // P                 # 4 i-chunks
K = D_IN                      # 8
HALF = T // 2                 # chunks per DMA half
W_COLS = T * N_PER * K        # 640
X_COLS = T * K                # 32
H_W = HALF * N_PER * K        # 320  W cols per half
H_X = HALF * K * 2            # 32   x cols per half (full x copy)
H_COLS = H_X + H_W            # 352  cols per half
TOT = 2 * H_COLS              # 704

USE_F32R = os.environ.get("DIGITCAPS_F32R", "1") == "1"

_built = None
last_results = None           # BassKernelResults of the most recent run


def _build_nc():
    import concourse.bass as bass
    import concourse.tile as tile
    from concourse import bacc, mybir

    nc = bacc.Bacc("TRN2", num_devices=N_CORES)
    inp = nc.dram_tensor("inp", (P, TOT), mybir.dt.float32, kind="ExternalInput")
    out = nc.dram_tensor("out", (1, N_PER), mybir.dt.float32, kind="ExternalOutput")

    f32 = mybir.dt.float32
    f32r = mybir.dt.float32r
    with tile.TileContext(nc) as tc, ExitStack() as ctx:
        pool = ctx.enter_context(tc.tile_pool(name="p", bufs=1))
        pspool = ctx.enter_context(tc.tile_pool(name="ps", bufs=1, space="PSUM"))

        buf = pool.tile([P, TOT], f32)
        # half 0 on the SP HWDGE ring, half 1 on the ACT ring — independent
        # FIFOs, so the transfers and completions overlap
        nc.sync.dma_start(out=buf[:, 0:H_COLS], in_=inp[:, 0:H_COLS])
        nc.scalar.dma_start(out=buf[:, H_COLS:TOT], in_=inp[:, H_COLS:TOT])

        # stationary 1/512 column (memset on DVE so the matmul's lhsT and rhs
        # deps ride the same semaphore — walrus fits one wait per compute op)
        ones = pool.tile([P, 1], f32)
        if USE_F32R:
            ones_raw = pool.tile([P, 1], f32)
            nc.vector.memset(ones_raw, 1.0 / N_IN)
            nc.vector.tensor_copy(ones.bitcast(f32r), ones_raw)
        else:
            nc.vector.memset(ones, 1.0 / N_IN)

        # T[p, t, n, k] = W[p, t, n, k] * x[p, t, k]; one TT per half, each
        # reading only its own half's DMA
        tmul = pool.tile([P, W_COLS], f32)
        for h in range(2):
            x_sl = buf[:, h * H_COLS : h * H_COLS + H_X]
            x_b = bass.AP(
                tensor=x_sl.tensor,
                offset=x_sl.offset + h * HALF * K,
                ap=[x_sl.ap[0], [K, HALF], [0, N_PER], [1, K]],
            )
            w_4d = buf[
                :, h * H_COLS + H_X : (h + 1) * H_COLS
            ].rearrange("p (t n k) -> p t n k", t=HALF, n=N_PER)
            t_4d = tmul[:, h * H_W : (h + 1) * H_W].rearrange(
                "p (t n k) -> p t n k", t=HALF, n=N_PER
            )
            if USE_F32R:
                t_4d = t_4d.bitcast(f32r)
            nc.vector.tensor_tensor(t_4d, w_4d, x_b, op=mybir.AluOpType.mult)

        # psum[0, n*K+k] = (1/512) * sum_{p,t} T[p, t, n, k]
        ps = pspool.tile([1, N_PER * K], f32)
        for t in range(T):
            lhsT = ones[:, 0:1]
            rhs = tmul[:, t * N_PER * K : (t + 1) * N_PER * K]
            if USE_F32R:
                lhsT = lhsT.bitcast(f32r)
                rhs = rhs.bitcast(f32r)
            nc.tensor.matmul(
                ps[0:1, :], lhsT=lhsT, rhs=rhs, start=(t == 0), stop=(t == T - 1)
            )

        # s[1, n] = sum_k psum[1, n, k]
        s = pool.tile([1, N_PER], f32)
        nc.vector.tensor_reduce(
            s,
            ps[0:1, :].rearrange("p (n k) -> p n k", n=N_PER),
            axis=mybir.AxisListType.X,
            op=mybir.AluOpType.add,
        )

        # squash: out = (s*sq) / ((1+sq) * (sqrt(sq+EPS)+EPS))
        # sq on DVE (not ACT) so no op needs waits on two different sems.
        eps_t = pool.tile([1, 1], f32)
        nc.vector.memset(eps_t, EPS)
        sq = pool.tile([1, N_PER], f32)
        nc.vector.tensor_mul(sq, s, s)
        r = pool.tile([1, N_PER], f32)
        nc.scalar.activation(
            r, sq, mybir.ActivationFunctionType.Sqrt, bias=eps_t[0:1, 0:1]
        )
        num = pool.tile([1, N_PER], f32)
        nc.vector.tensor_mul(num, s, sq)
        d1 = pool.tile([1, N_PER], f32)
        nc.vector.tensor_scalar_add(d1, sq, 1.0)
        d2 = pool.tile([1, N_PER], f32)
        nc.vector.tensor_scalar_add(d2, r, EPS)
        den = pool.tile([1, N_PER], f32)
        nc.vector.tensor_mul(den, d1, d2)
        rec = pool.tile([1, N_PER], f32)
        nc.vector.reciprocal(rec, den)
        q = pool.tile([1, N_PER], f32)
        nc.vector.tensor_mul(q, num, rec)

        nc.sync.dma_start(out=out[:, :], in_=q)
    nc.finalize()
    return nc


def kernel(x, W):
    global _built, last_results
    from concourse.bass_utils import run_bass_kernel_spmd

    if _built is None:
        _built = _build_nc()
    nc = _built

    x = np.ascontiguousarray(np.asarray(x, dtype=np.float32))
    W = np.ascontiguousarray(np.asarray(W, dtype=np.float32))

    # xr[p, t*K + k] = x[t*128 + p, k]
    xr = x.reshape(T, P, K).transpose(1, 0, 2).reshape(P, X_COLS)
    base = np.empty((P, TOT), dtype=np.float32)
    base[:, 0:H_X] = xr
    base[:, H_COLS : H_COLS + H_X] = xr

    in_maps = []
    for c in range(N_CORES):
        Wc = W[0][:, :, D_PER * c : D_PER * (c + 1), :]     # (512, 10, 2, 8)
        Wr = (
            Wc.reshape(T, P, N_OUT, D_PER, K)
            .transpose(1, 0, 2, 3, 4)
            .reshape(P, W_COLS)
        )
        buf = base.copy()
        buf[:, H_X:H_COLS] = Wr[:, 0:H_W]
        buf[:, H_COLS + H_X : TOT] = Wr[:, H_W:W_COLS]
        in_maps.append({"inp": buf})

    res = run_bass_kernel_spmd(nc, in_maps, core_ids=list(range(N_CORES)))
    last_results = res

    v = np.zeros((N_OUT, D_OUT), dtype=np.float32)
    for c in range(N_CORES):
        v[:, D_PER * c : D_PER * (c + 1)] = res.results[c]["out"].reshape(
            N_OUT, D_PER
        )
    return v.reshape(1, 1, N_OUT, D_OUT, 1)
